# revision 12
# baseline (speedup 1.0000x reference)
"""3x3 median blur (replicate padding) on Trainium2, 8-core data parallel.

Problem: noised_image [32,3,512,512] f32 -> median-blurred; cover_image passthrough.

Strategy:
- Shard batch across 8 NeuronCores: 4 images (12 channel-planes) per core.
- Host-side edge-pad each 512x512 plane to 514x514 AND convert f32 -> bf16.
  bf16 halves DMA traffic and doubles Vector-engine throughput (2x_1p perf
  mode for packed 16-bit operands); min/max on bf16 selects the bf16-rounded
  true median (monotone rounding commutes with order statistics), so error is
  bounded by bf16 rounding (~2^-8 relative), far inside the 2e-2 gate.
- The Vector engine (DVE) is the ONLY engine that can run binary min/max
  elementwise ops on this toolchain (neuronxcc rejects TensorTensor on the
  Pool/GPSIMD engine; ACT/PE have no binary elementwise path), so the kernel
  is a single-DVE pipeline with DMA overlapped on the sync engine.
- "Vertical pack-2" strip layout: one strip = TWO planes; plane 0 lives in
  partitions 0..63, plane 1 in partitions 64..127. Partition p holds padded
  rows 8p..8p+9 (10 rows x 514 bf16 free dim), giving 8 output rows per
  partition. Halves per-op instruction overhead vs the 4-row layout and
  shrinks the pair stage (5 pair-rows per 8 output rows vs 5 per 4).
- Exact median-of-9 via a min/max network with even-pair sharing: vertical
  pairs are computed only for even row offsets {(0,1),(2,3),..}; window w
  uses pair (w,w+1) for even w and pair (w+1,w+2) for odd w, so each pair
  serves two windows. Then the standard sorted-column merge:
  med3(max3(lo), med3(mid), min3(hi)) with horizontal sliding reuse.
- Raw Bass program (explicit semaphores), double-buffered input/output
  tiles; parity-split DMA-completion semaphores so two in-flight DMAs never
  share a semaphore value a waiter could confuse.
"""
import sys
sys.path.insert(0, '/opt/trn_rl_repo')
from contextlib import ExitStack
import numpy as np
import ml_dtypes

import concourse.bass as bass
import concourse.mybir as mybir
import bass_rust
from concourse import bass_utils

BF16 = mybir.dt.bfloat16
MIN = mybir.AluOpType.min
MAX = mybir.AluOpType.max

N_CORES = 8
N_CH = 12          # channel-planes per core (4 images x 3 channels)
H = W = 512
HP = WP = 514      # host-padded plane
R = 8              # output rows per partition (2 planes x 64 partitions)
PAIRS = R // 2 + 1  # even-offset vertical pairs per partition


def _mk_ap(base, dims, offset):
    c = base.copy()
    c.ap = bass_rust.VecI64Pair(dims)
    c.offset = offset
    return c


def _build_nc(n_ch=N_CH, reps=1):
    assert n_ch % 2 == 0
    nc = bass.Bass("TRN2")
    x = nc.dram_tensor("x", [n_ch, HP, WP], BF16, kind="ExternalInput")
    y = nc.dram_tensor("y", [n_ch, W, W], BF16, kind="ExternalOutput")
    n_strips = (n_ch // 2) * reps

    dve_done_xs = [0] * n_strips    # xs consumers done (buffer reusable)
    dve_done_out = [0] * n_strips   # out tile ready for DMA

    with ExitStack() as ctx:
        xs = [ctx.enter_context(nc.sbuf_tensor(f"xs{i}", [128, 10, WP], BF16)) for i in range(2)]
        out = [ctx.enter_context(nc.sbuf_tensor(f"outb{i}", [128, R, W], BF16)) for i in range(2)]
        pmn = ctx.enter_context(nc.sbuf_tensor("pmn", [128, PAIRS, WP], BF16))
        pmx = ctx.enter_context(nc.sbuf_tensor("pmx", [128, PAIRS, WP], BF16))
        lo3 = ctx.enter_context(nc.sbuf_tensor("lo3", [128, R, WP], BF16))
        hi3 = ctx.enter_context(nc.sbuf_tensor("hi3", [128, R, WP], BF16))
        tt = ctx.enter_context(nc.sbuf_tensor("tt", [128, R, WP], BF16))
        mid3 = ctx.enter_context(nc.sbuf_tensor("mid3", [128, R, WP], BF16))
        mlo = ctx.enter_context(nc.sbuf_tensor("mlo", [128, R, W + 1], BF16))
        mhi = ctx.enter_context(nc.sbuf_tensor("mhi", [128, R, W + 1], BF16))
        qmn = ctx.enter_context(nc.sbuf_tensor("qmn", [128, R, W + 1], BF16))
        qmx = ctx.enter_context(nc.sbuf_tensor("qmx", [128, R, W + 1], BF16))
        A = ctx.enter_context(nc.sbuf_tensor("A", [128, R, W], BF16))
        C = ctx.enter_context(nc.sbuf_tensor("C", [128, R, W], BF16))
        u = ctx.enter_context(nc.sbuf_tensor("u", [128, R, W], BF16))
        B = ctx.enter_context(nc.sbuf_tensor("B", [128, R, W], BF16))
        fmn = ctx.enter_context(nc.sbuf_tensor("fmn", [128, R, W], BF16))
        fmx = ctx.enter_context(nc.sbuf_tensor("fmx", [128, R, W], BF16))
        v = ctx.enter_context(nc.sbuf_tensor("v", [128, R, W], BF16))

        # Parity-split DMA-completion semaphores (see module docstring).
        sem_in = [ctx.enter_context(nc.semaphore(name=f"sem_in{i}")) for i in range(2)]
        sem_out = [ctx.enter_context(nc.semaphore(name=f"sem_out{i}")) for i in range(2)]
        sem_dve = ctx.enter_context(nc.semaphore())

        def in_ready(i):    # both plane-DMAs of strip i landed
            return sem_in[i % 2], 32 * (i // 2 + 1)

        def out_done(i):    # both plane-DMAs of strip i's output completed
            return sem_out[i % 2], 32 * (i // 2 + 1)

        block = ctx.enter_context(nc.Block())

        @block.vector
        def _(vector):
            cnt = 0

            def inc(t):
                nonlocal cnt
                t.then_inc(sem_dve, 1)
                cnt += 1

            for i in range(n_strips):
                xv = xs[i % 2]
                ov = out[i % 2]
                vector.wait_ge(*in_ready(i))
                # vertical pairs at even row offsets: (0,1),(2,3),...,(8,9)
                inc(vector.tensor_tensor(pmn[:, :, :], xv[:, 0:10:2, :], xv[:, 1:10:2, :], MIN))
                inc(vector.tensor_tensor(pmx[:, :, :], xv[:, 0:10:2, :], xv[:, 1:10:2, :], MAX))
                # window w: even w uses pair w/2 with c=row w+2;
                #           odd  w uses pair (w+1)/2 with c=row w
                for (sl, pr, cr) in (
                    (slice(0, R, 2), slice(0, 4), slice(2, 10, 2)),   # even windows
                    (slice(1, R, 2), slice(1, 5), slice(1, 9, 2)),    # odd windows
                ):
                    inc(vector.tensor_tensor(lo3[:, sl, :], pmn[:, pr, :], xv[:, cr, :], MIN))
                    inc(vector.tensor_tensor(hi3[:, sl, :], pmx[:, pr, :], xv[:, cr, :], MAX))
                    inc(vector.tensor_tensor(tt[:, sl, :], pmx[:, pr, :], xv[:, cr, :], MIN))
                    inc(vector.tensor_tensor(mid3[:, sl, :], pmn[:, pr, :], tt[:, sl, :], MAX))
                dve_done_xs[i] = cnt
                # horizontal stage
                inc(vector.tensor_tensor(mlo[:, :, :], lo3[:, :, 0:W + 1], lo3[:, :, 1:WP], MAX))
                inc(vector.tensor_tensor(mhi[:, :, :], hi3[:, :, 0:W + 1], hi3[:, :, 1:WP], MIN))
                inc(vector.tensor_tensor(qmn[:, :, :], mid3[:, :, 0:W + 1], mid3[:, :, 1:WP], MIN))
                inc(vector.tensor_tensor(qmx[:, :, :], mid3[:, :, 0:W + 1], mid3[:, :, 1:WP], MAX))
                inc(vector.tensor_tensor(A[:, :, :], mlo[:, :, 0:W], lo3[:, :, 2:WP], MAX))
                inc(vector.tensor_tensor(C[:, :, :], mhi[:, :, 0:W], hi3[:, :, 2:WP], MIN))
                inc(vector.tensor_tensor(u[:, :, :], qmx[:, :, 0:W], mid3[:, :, 2:WP], MIN))
                inc(vector.tensor_tensor(B[:, :, :], qmn[:, :, 0:W], u[:, :, :], MAX))
                # final med3(A, B, C)
                inc(vector.tensor_tensor(fmn[:, :, :], A[:, :, :], B[:, :, :], MIN))
                inc(vector.tensor_tensor(fmx[:, :, :], A[:, :, :], B[:, :, :], MAX))
                inc(vector.tensor_tensor(v[:, :, :], fmx[:, :, :], C[:, :, :], MIN))
                if i >= 2:
                    vector.wait_ge(*out_done(i - 2))
                inc(vector.tensor_tensor(ov[:, :, :], fmn[:, :, :], v[:, :, :], MAX))
                dve_done_out[i] = cnt

        @block.sync
        def _(sync):
            def in_dma(i):
                for pl in range(2):
                    ch = (2 * i + pl) % n_ch
                    src = _mk_ap(x[ch], [[R * WP, 64], [WP, 10], [1, WP]], ch * HP * WP)
                    dst = xs[i % 2][64 * pl:64 * (pl + 1), :, :]
                    sync.dma_start(dst, src).then_inc(sem_in[i % 2], 16)

            def out_dma(i):
                for pl in range(2):
                    ch = (2 * i + pl) % n_ch
                    dst = y[ch].rearrange("(p r) w -> p r w", r=R)
                    src = out[i % 2][64 * pl:64 * (pl + 1), :, :]
                    sync.dma_start(dst, src).then_inc(sem_out[i % 2], 16)

            for i in range(n_strips):
                if i >= 2:
                    sync.wait_ge(sem_dve, dve_done_xs[i - 2])
                in_dma(i)
                if i >= 1:
                    sync.wait_ge(sem_dve, dve_done_out[i - 1])
                    out_dma(i - 1)
            sync.wait_ge(sem_dve, dve_done_out[n_strips - 1])
            out_dma(n_strips - 1)
    return nc


_NC_CACHE = {}


def _get_nc():
    if "nc" not in _NC_CACHE:
        _NC_CACHE["nc"] = _build_nc()
    return _NC_CACHE["nc"]


def kernel(noised_image, cover_image):
    noised_image = np.ascontiguousarray(noised_image, dtype=np.float32)
    nc = _get_nc()
    per = noised_image.shape[0] // N_CORES  # 4 images per core
    in_maps = []
    for c in range(N_CORES):
        shard = noised_image[c * per:(c + 1) * per].reshape(N_CH, H, W)
        padded = np.pad(shard, ((0, 0), (1, 1), (1, 1)), mode='edge')
        in_maps.append({"x": np.ascontiguousarray(padded.astype(ml_dtypes.bfloat16))})
    res = bass_utils.run_bass_kernel_spmd(nc, in_maps, core_ids=list(range(N_CORES)))
    blurred = np.stack([np.asarray(r["y"]).astype(np.float32).reshape(per, 3, H, W)
                        for r in res.results])
    blurred = blurred.reshape(noised_image.shape)
    return (blurred, cover_image)


# revision 13
# speedup vs baseline: 1.0623x; 1.0623x over previous
"""3x3 median blur (replicate padding) on Trainium2, 8-core data parallel.

Problem: noised_image [32,3,512,512] f32 -> median-blurred; cover_image passthrough.

Strategy:
- Shard batch across 8 NeuronCores: 4 images (12 channel-planes) per core.
- Host-side edge-pad each 512x512 plane to 514x514 AND convert f32 -> bf16.
  bf16 halves DMA traffic and doubles Vector-engine throughput (2x_1p perf
  mode for packed 16-bit operands); min/max on bf16 selects the bf16-rounded
  true median (monotone rounding commutes with order statistics), so error is
  bounded by bf16 rounding (~2^-8 relative), far inside the 2e-2 gate.
- The Vector engine (DVE) is the ONLY engine that can run binary min/max
  elementwise ops on this toolchain (neuronxcc rejects TensorTensor on the
  Pool/GPSIMD engine; ACT/PE have no binary elementwise path), so the kernel
  is a single-DVE pipeline with DMA overlapped on the sync engine.
- "Vertical pack-2" strip layout: one strip = TWO planes; plane 0 lives in
  partitions 0..63, plane 1 in partitions 64..127. Partition p holds padded
  rows 8p..8p+9 (10 rows x 514 bf16 free dim), giving 8 output rows per
  partition. Halves per-op instruction overhead vs the 4-row layout and
  shrinks the pair stage (5 pair-rows per 8 output rows vs 5 per 4).
- Exact median-of-9 via a min/max network with even-pair sharing: vertical
  pairs are computed only for even row offsets {(0,1),(2,3),..}; window w
  uses pair (w,w+1) for even w and pair (w+1,w+2) for odd w, so each pair
  serves two windows. Then the standard sorted-column merge:
  med3(max3(lo), med3(mid), min3(hi)) with horizontal sliding reuse.
- Raw Bass program (explicit semaphores), double-buffered input/output
  tiles; parity-split DMA-completion semaphores so two in-flight DMAs never
  share a semaphore value a waiter could confuse.
"""
import sys
sys.path.insert(0, '/opt/trn_rl_repo')
from contextlib import ExitStack
import numpy as np
import ml_dtypes

import concourse.bass as bass
import concourse.mybir as mybir
import bass_rust
from concourse import bass_utils

BF16 = mybir.dt.bfloat16
MIN = mybir.AluOpType.min
MAX = mybir.AluOpType.max

N_CORES = 8
N_CH = 12          # channel-planes per core (4 images x 3 channels)
H = W = 512
HP = WP = 514      # host-padded plane
R = 8              # output rows per partition (2 planes x 64 partitions)
PAIRS = R // 2 + 1  # even-offset vertical pairs per partition


def _mk_ap(base, dims, offset):
    c = base.copy()
    c.ap = bass_rust.VecI64Pair(dims)
    c.offset = offset
    return c


def _build_nc(n_ch=N_CH, reps=1):
    assert n_ch % 2 == 0
    nc = bass.Bass("TRN2")
    x = nc.dram_tensor("x", [n_ch, HP, WP], BF16, kind="ExternalInput")
    y = nc.dram_tensor("y", [n_ch, W, W], BF16, kind="ExternalOutput")
    n_strips = (n_ch // 2) * reps

    dve_done_xs = [0] * n_strips    # xs consumers done (buffer reusable)
    dve_done_out = [0] * n_strips   # out tile ready for DMA

    with ExitStack() as ctx:
        xs = [ctx.enter_context(nc.sbuf_tensor(f"xs{i}", [128, 10, WP], BF16)) for i in range(2)]
        out = [ctx.enter_context(nc.sbuf_tensor(f"outb{i}", [128, R, W], BF16)) for i in range(2)]
        pmn = ctx.enter_context(nc.sbuf_tensor("pmn", [128, PAIRS, WP], BF16))
        pmx = ctx.enter_context(nc.sbuf_tensor("pmx", [128, PAIRS, WP], BF16))
        lo3 = ctx.enter_context(nc.sbuf_tensor("lo3", [128, R, WP], BF16))
        hi3 = ctx.enter_context(nc.sbuf_tensor("hi3", [128, R, WP], BF16))
        tt = ctx.enter_context(nc.sbuf_tensor("tt", [128, R, WP], BF16))
        mid3 = ctx.enter_context(nc.sbuf_tensor("mid3", [128, R, WP], BF16))
        mlo = ctx.enter_context(nc.sbuf_tensor("mlo", [128, R, W + 1], BF16))
        mhi = ctx.enter_context(nc.sbuf_tensor("mhi", [128, R, W + 1], BF16))
        qmn = ctx.enter_context(nc.sbuf_tensor("qmn", [128, R, W + 1], BF16))
        qmx = ctx.enter_context(nc.sbuf_tensor("qmx", [128, R, W + 1], BF16))
        A = ctx.enter_context(nc.sbuf_tensor("A", [128, R, W], BF16))
        C = ctx.enter_context(nc.sbuf_tensor("C", [128, R, W], BF16))
        u = ctx.enter_context(nc.sbuf_tensor("u", [128, R, W], BF16))
        B = ctx.enter_context(nc.sbuf_tensor("B", [128, R, W], BF16))
        fmn = ctx.enter_context(nc.sbuf_tensor("fmn", [128, R, W], BF16))
        fmx = ctx.enter_context(nc.sbuf_tensor("fmx", [128, R, W], BF16))
        v = ctx.enter_context(nc.sbuf_tensor("v", [128, R, W], BF16))

        # Parity-split DMA-completion semaphores (see module docstring).
        sem_in = [ctx.enter_context(nc.semaphore(name=f"sem_in{i}")) for i in range(2)]
        sem_out = [ctx.enter_context(nc.semaphore(name=f"sem_out{i}")) for i in range(2)]
        sem_dve = ctx.enter_context(nc.semaphore())

        def in_ready(i):    # both plane-DMAs of strip i landed
            return sem_in[i % 2], 32 * (i // 2 + 1)

        def out_done(i):    # both plane-DMAs of strip i's output completed
            return sem_out[i % 2], 32 * (i // 2 + 1)

        block = ctx.enter_context(nc.Block())

        @block.vector
        def _(vector):
            cnt = 0

            def inc(t):
                nonlocal cnt
                t.then_inc(sem_dve, 1)
                cnt += 1

            for i in range(n_strips):
                xv = xs[i % 2]
                ov = out[i % 2]
                vector.wait_ge(*in_ready(i))
                # vertical pairs at even row offsets: (0,1),(2,3),...,(8,9)
                inc(vector.tensor_tensor(pmn[:, :, :], xv[:, 0:10:2, :], xv[:, 1:10:2, :], MIN))
                inc(vector.tensor_tensor(pmx[:, :, :], xv[:, 0:10:2, :], xv[:, 1:10:2, :], MAX))
                # window w: even w uses pair w/2 with c=row w+2;
                #           odd  w uses pair (w+1)/2 with c=row w
                for (sl, pr, cr) in (
                    (slice(0, R, 2), slice(0, 4), slice(2, 10, 2)),   # even windows
                    (slice(1, R, 2), slice(1, 5), slice(1, 9, 2)),    # odd windows
                ):
                    inc(vector.tensor_tensor(lo3[:, sl, :], pmn[:, pr, :], xv[:, cr, :], MIN))
                    inc(vector.tensor_tensor(hi3[:, sl, :], pmx[:, pr, :], xv[:, cr, :], MAX))
                    inc(vector.tensor_tensor(tt[:, sl, :], pmx[:, pr, :], xv[:, cr, :], MIN))
                    inc(vector.tensor_tensor(mid3[:, sl, :], pmn[:, pr, :], tt[:, sl, :], MAX))
                dve_done_xs[i] = cnt

                # Horizontal stage, split into column halves (~2048 elems per
                # instruction) and emitted left/right interleaved: halves are
                # mutually independent, so no instruction directly follows its
                # producer, and each stays in the DVE's fast small-op regime.
                def half(op, dst, s0, s1, alu):
                    # dst/s0/s1 are (tensor, col_lo) pairs; split at column M
                    M = 257 if op == 'w1' else 256
                    for lo, hi in ((0, M), (M, (W + 1) if op == 'w1' else W)):
                        d, db = dst
                        a0, a0b = s0
                        a1, a1b = s1
                        inc(vector.tensor_tensor(
                            d[:, :, db + lo: db + hi],
                            a0[:, :, a0b + lo: a0b + hi],
                            a1[:, :, a1b + lo: a1b + hi], alu))

                half('w1', (mlo, 0), (lo3, 0), (lo3, 1), MAX)
                half('w1', (mhi, 0), (hi3, 0), (hi3, 1), MIN)
                half('w1', (qmn, 0), (mid3, 0), (mid3, 1), MIN)
                half('w1', (qmx, 0), (mid3, 0), (mid3, 1), MAX)
                half('w0', (A, 0), (mlo, 0), (lo3, 2), MAX)
                half('w0', (C, 0), (mhi, 0), (hi3, 2), MIN)
                half('w0', (u, 0), (qmx, 0), (mid3, 2), MIN)
                half('w0', (B, 0), (qmn, 0), (u, 0), MAX)
                # final med3(A, B, C)
                half('w0', (fmn, 0), (A, 0), (B, 0), MIN)
                half('w0', (fmx, 0), (A, 0), (B, 0), MAX)
                half('w0', (v, 0), (fmx, 0), (C, 0), MIN)
                if i >= 2:
                    vector.wait_ge(*out_done(i - 2))
                half('w0', (ov, 0), (fmn, 0), (v, 0), MAX)
                dve_done_out[i] = cnt

        @block.sync
        def _(sync):
            def in_dma(i):
                for pl in range(2):
                    ch = (2 * i + pl) % n_ch
                    src = _mk_ap(x[ch], [[R * WP, 64], [WP, 10], [1, WP]], ch * HP * WP)
                    dst = xs[i % 2][64 * pl:64 * (pl + 1), :, :]
                    sync.dma_start(dst, src).then_inc(sem_in[i % 2], 16)

            def out_dma(i):
                for pl in range(2):
                    ch = (2 * i + pl) % n_ch
                    dst = y[ch].rearrange("(p r) w -> p r w", r=R)
                    src = out[i % 2][64 * pl:64 * (pl + 1), :, :]
                    sync.dma_start(dst, src).then_inc(sem_out[i % 2], 16)

            for i in range(n_strips):
                if i >= 2:
                    sync.wait_ge(sem_dve, dve_done_xs[i - 2])
                in_dma(i)
                if i >= 1:
                    sync.wait_ge(sem_dve, dve_done_out[i - 1])
                    out_dma(i - 1)
            sync.wait_ge(sem_dve, dve_done_out[n_strips - 1])
            out_dma(n_strips - 1)
    return nc


_NC_CACHE = {}


def _get_nc():
    if "nc" not in _NC_CACHE:
        _NC_CACHE["nc"] = _build_nc()
    return _NC_CACHE["nc"]


def kernel(noised_image, cover_image):
    noised_image = np.ascontiguousarray(noised_image, dtype=np.float32)
    nc = _get_nc()
    per = noised_image.shape[0] // N_CORES  # 4 images per core
    in_maps = []
    for c in range(N_CORES):
        shard = noised_image[c * per:(c + 1) * per].reshape(N_CH, H, W)
        padded = np.pad(shard, ((0, 0), (1, 1), (1, 1)), mode='edge')
        in_maps.append({"x": np.ascontiguousarray(padded.astype(ml_dtypes.bfloat16))})
    res = bass_utils.run_bass_kernel_spmd(nc, in_maps, core_ids=list(range(N_CORES)))
    blurred = np.stack([np.asarray(r["y"]).astype(np.float32).reshape(per, 3, H, W)
                        for r in res.results])
    blurred = blurred.reshape(noised_image.shape)
    return (blurred, cover_image)


# revision 14
# speedup vs baseline: 1.1418x; 1.0748x over previous
"""3x3 median blur (replicate padding) on Trainium2, 8-core data parallel.

Problem: noised_image [32,3,512,512] f32 -> median-blurred; cover_image passthrough.

Strategy:
- Shard batch across 8 NeuronCores: 4 images (12 channel-planes) per core.
- Host-side edge-pad each 512x512 plane to 514x514 AND convert f32 -> bf16.
  bf16 halves DMA traffic and doubles Vector-engine throughput (2x_1p perf
  mode for packed 16-bit operands); min/max on bf16 selects the bf16-rounded
  true median (monotone rounding commutes with order statistics), so error is
  bounded by bf16 rounding (~2^-8 relative), far inside the 2e-2 gate.
- The Vector engine (DVE) is the ONLY engine that can run binary min/max
  elementwise ops on this toolchain (neuronxcc rejects TensorTensor on the
  Pool/GPSIMD engine; ACT/PE have no binary elementwise path), so the kernel
  is a single-DVE pipeline with DMA overlapped on the sync engine.
- "Vertical pack-2" strip layout: one strip = TWO planes; plane 0 lives in
  partitions 0..63, plane 1 in partitions 64..127. Partition p holds padded
  rows 8p..8p+9 (10 rows x 514 bf16 free dim), giving 8 output rows per
  partition. Halves per-op instruction overhead vs the 4-row layout and
  shrinks the pair stage (5 pair-rows per 8 output rows vs 5 per 4).
- Exact median-of-9 via a min/max network with even-pair sharing: vertical
  pairs are computed only for even row offsets {(0,1),(2,3),..}; window w
  uses pair (w,w+1) for even w and pair (w+1,w+2) for odd w, so each pair
  serves two windows. Then the standard sorted-column merge:
  med3(max3(lo), med3(mid), min3(hi)) with horizontal sliding reuse.
- Raw Bass program (explicit semaphores), double-buffered input/output
  tiles; parity-split DMA-completion semaphores so two in-flight DMAs never
  share a semaphore value a waiter could confuse.
"""
import sys
sys.path.insert(0, '/opt/trn_rl_repo')
from contextlib import ExitStack
import numpy as np
import ml_dtypes

import concourse.bass as bass
import concourse.mybir as mybir
import bass_rust
from concourse import bass_utils

BF16 = mybir.dt.bfloat16
MIN = mybir.AluOpType.min
MAX = mybir.AluOpType.max

N_CORES = 8
N_CH = 12          # channel-planes per core (4 images x 3 channels)
H = W = 512
HP = WP = 514      # host-padded plane
R = 8              # output rows per partition (2 planes x 64 partitions)
PAIRS = R // 2 + 1  # even-offset vertical pairs per partition


def _mk_ap(base, dims, offset):
    c = base.copy()
    c.ap = bass_rust.VecI64Pair(dims)
    c.offset = offset
    return c


def _build_nc(n_ch=N_CH, reps=1):
    assert n_ch % 2 == 0
    nc = bass.Bass("TRN2")
    x = nc.dram_tensor("x", [n_ch, HP, WP], BF16, kind="ExternalInput")
    y = nc.dram_tensor("y", [n_ch, W, W], BF16, kind="ExternalOutput")
    n_strips = (n_ch // 2) * reps

    dve_done_xs = [0] * n_strips    # xs consumers done (buffer reusable)
    dve_done_out = [0] * n_strips   # out tile ready for DMA

    with ExitStack() as ctx:
        xs = [ctx.enter_context(nc.sbuf_tensor(f"xs{i}", [128, 10, WP], BF16)) for i in range(2)]
        out = [ctx.enter_context(nc.sbuf_tensor(f"outb{i}", [128, R, W], BF16)) for i in range(2)]
        pmn = ctx.enter_context(nc.sbuf_tensor("pmn", [128, PAIRS, WP], BF16))
        pmx = ctx.enter_context(nc.sbuf_tensor("pmx", [128, PAIRS, WP], BF16))
        lo3 = ctx.enter_context(nc.sbuf_tensor("lo3", [128, R, WP], BF16))
        hi3 = ctx.enter_context(nc.sbuf_tensor("hi3", [128, R, WP], BF16))
        tt = ctx.enter_context(nc.sbuf_tensor("tt", [128, R, WP], BF16))
        mid3 = ctx.enter_context(nc.sbuf_tensor("mid3", [128, R, WP], BF16))
        mlo = ctx.enter_context(nc.sbuf_tensor("mlo", [128, R, W + 1], BF16))
        mhi = ctx.enter_context(nc.sbuf_tensor("mhi", [128, R, W + 1], BF16))
        qmn = ctx.enter_context(nc.sbuf_tensor("qmn", [128, R, W + 1], BF16))
        qmx = ctx.enter_context(nc.sbuf_tensor("qmx", [128, R, W + 1], BF16))
        A = ctx.enter_context(nc.sbuf_tensor("A", [128, R, W], BF16))
        C = ctx.enter_context(nc.sbuf_tensor("C", [128, R, W], BF16))
        u = ctx.enter_context(nc.sbuf_tensor("u", [128, R, W], BF16))
        B = ctx.enter_context(nc.sbuf_tensor("B", [128, R, W], BF16))
        fmn = ctx.enter_context(nc.sbuf_tensor("fmn", [128, R, W], BF16))
        fmx = ctx.enter_context(nc.sbuf_tensor("fmx", [128, R, W], BF16))
        v = ctx.enter_context(nc.sbuf_tensor("v", [128, R, W], BF16))

        # Parity-split DMA-completion semaphores (see module docstring).
        sem_in = [ctx.enter_context(nc.semaphore(name=f"sem_in{i}")) for i in range(2)]
        sem_out = [ctx.enter_context(nc.semaphore(name=f"sem_out{i}")) for i in range(2)]
        sem_dve = ctx.enter_context(nc.semaphore())

        def in_ready(i):    # both plane-DMAs of strip i landed
            return sem_in[i % 2], 32 * (i // 2 + 1)

        def out_done(i):    # both plane-DMAs of strip i's output completed
            return sem_out[i % 2], 32 * (i // 2 + 1)

        block = ctx.enter_context(nc.Block())

        @block.vector
        def _(vector):
            cnt = 0

            def inc(t):
                nonlocal cnt
                t.then_inc(sem_dve, 1)
                cnt += 1

            for i in range(n_strips):
                xv = xs[i % 2]
                ov = out[i % 2]
                vector.wait_ge(*in_ready(i))
                # vertical pairs at even row offsets: (0,1),(2,3),...,(8,9)
                inc(vector.tensor_tensor(pmn[:, :, :], xv[:, 0:10:2, :], xv[:, 1:10:2, :], MIN))
                inc(vector.tensor_tensor(pmx[:, :, :], xv[:, 0:10:2, :], xv[:, 1:10:2, :], MAX))
                # window w: even w uses pair w/2 with c=row w+2;
                #           odd  w uses pair (w+1)/2 with c=row w
                for (sl, pr, cr) in (
                    (slice(0, R, 2), slice(0, 4), slice(2, 10, 2)),   # even windows
                    (slice(1, R, 2), slice(1, 5), slice(1, 9, 2)),    # odd windows
                ):
                    inc(vector.tensor_tensor(lo3[:, sl, :], pmn[:, pr, :], xv[:, cr, :], MIN))
                    inc(vector.tensor_tensor(hi3[:, sl, :], pmx[:, pr, :], xv[:, cr, :], MAX))
                    inc(vector.tensor_tensor(tt[:, sl, :], pmx[:, pr, :], xv[:, cr, :], MIN))
                    inc(vector.tensor_tensor(mid3[:, sl, :], pmn[:, pr, :], tt[:, sl, :], MAX))
                dve_done_xs[i] = cnt
                # horizontal stage
                inc(vector.tensor_tensor(mlo[:, :, :], lo3[:, :, 0:W + 1], lo3[:, :, 1:WP], MAX))
                inc(vector.tensor_tensor(mhi[:, :, :], hi3[:, :, 0:W + 1], hi3[:, :, 1:WP], MIN))
                inc(vector.tensor_tensor(qmn[:, :, :], mid3[:, :, 0:W + 1], mid3[:, :, 1:WP], MIN))
                inc(vector.tensor_tensor(qmx[:, :, :], mid3[:, :, 0:W + 1], mid3[:, :, 1:WP], MAX))
                inc(vector.tensor_tensor(A[:, :, :], mlo[:, :, 0:W], lo3[:, :, 2:WP], MAX))
                inc(vector.tensor_tensor(C[:, :, :], mhi[:, :, 0:W], hi3[:, :, 2:WP], MIN))
                inc(vector.tensor_tensor(u[:, :, :], qmx[:, :, 0:W], mid3[:, :, 2:WP], MIN))
                inc(vector.tensor_tensor(B[:, :, :], qmn[:, :, 0:W], u[:, :, :], MAX))
                # final med3(A, B, C)
                inc(vector.tensor_tensor(fmn[:, :, :], A[:, :, :], B[:, :, :], MIN))
                inc(vector.tensor_tensor(fmx[:, :, :], A[:, :, :], B[:, :, :], MAX))
                inc(vector.tensor_tensor(v[:, :, :], fmx[:, :, :], C[:, :, :], MIN))
                if i >= 2:
                    vector.wait_ge(*out_done(i - 2))
                inc(vector.tensor_tensor(ov[:, :, :], fmn[:, :, :], v[:, :, :], MAX))
                dve_done_out[i] = cnt

        @block.sync
        def _(sync):
            def in_dma(i):
                for pl in range(2):
                    ch = (2 * i + pl) % n_ch
                    src = _mk_ap(x[ch], [[R * WP, 64], [WP, 10], [1, WP]], ch * HP * WP)
                    dst = xs[i % 2][64 * pl:64 * (pl + 1), :, :]
                    sync.dma_start(dst, src).then_inc(sem_in[i % 2], 16)

            def out_dma(i):
                for pl in range(2):
                    ch = (2 * i + pl) % n_ch
                    dst = y[ch].rearrange("(p r) w -> p r w", r=R)
                    src = out[i % 2][64 * pl:64 * (pl + 1), :, :]
                    sync.dma_start(dst, src).then_inc(sem_out[i % 2], 16)

            for i in range(n_strips):
                if i >= 2:
                    sync.wait_ge(sem_dve, dve_done_xs[i - 2])
                in_dma(i)
                if i >= 1:
                    sync.wait_ge(sem_dve, dve_done_out[i - 1])
                    out_dma(i - 1)
            sync.wait_ge(sem_dve, dve_done_out[n_strips - 1])
            out_dma(n_strips - 1)
    return nc


_NC_CACHE = {}


def _get_nc():
    if "nc" not in _NC_CACHE:
        _NC_CACHE["nc"] = _build_nc()
    return _NC_CACHE["nc"]


def kernel(noised_image, cover_image):
    noised_image = np.ascontiguousarray(noised_image, dtype=np.float32)
    nc = _get_nc()
    per = noised_image.shape[0] // N_CORES  # 4 images per core
    in_maps = []
    for c in range(N_CORES):
        shard = noised_image[c * per:(c + 1) * per].reshape(N_CH, H, W)
        padded = np.pad(shard, ((0, 0), (1, 1), (1, 1)), mode='edge')
        in_maps.append({"x": np.ascontiguousarray(padded.astype(ml_dtypes.bfloat16))})
    res = bass_utils.run_bass_kernel_spmd(nc, in_maps, core_ids=list(range(N_CORES)))
    blurred = np.stack([np.asarray(r["y"]).astype(np.float32).reshape(per, 3, H, W)
                        for r in res.results])
    blurred = blurred.reshape(noised_image.shape)
    return (blurred, cover_image)
